# revision 1
# baseline (speedup 1.0000x reference)
"""ContactMapLoss Trainium2 kernel.

Data-parallel over batch B=8 (one NeuronCore per batch element).

Per core, for batch b:
  p1 = v1[b][rid_to_vid.ravel()]  (3000 pts), p2 likewise (host-side gather).
  The PE computes H[i,j] = n2_j - 2<p1_i, p2_j> via a K=4 augmented matmul
  (rows (-2x,-2y,-2z,1) x cols (x,y,z,|p|^2)).  ScalarE then applies
  relu(H + n1_i) (per-partition bias) while downcasting PSUM fp32 -> SBUF
  fp16; VectorE min-reduces each 40-wide region window (fp16 tensor_tensor
  min tree at 2x mode).  Since sqrt is monotonic, min(d)^2 == min(d^2), so
  no sqrt is needed anywhere.  A second pass with swapped operands gives the
  other min direction.  An indicator matmul (E^T @ mins) accumulates
  per-region-pair sums in PSUM, and a masked (cmap) reduction produces the
  scalar loss.
"""

import numpy as np

B, V, R, M = 8, 10475, 75, 40
NR = R * M            # 3000 gathered points
NB = 24               # 128-row blocks
RP = NB * 128         # 3072 padded rows
CW = 440              # PSUM chunk width (11 regions of 40)
CH = 7                # chunks per block
CPAD = CH * CW        # 3080 padded columns (77 regions)
RPAD = CPAD // M      # 77

# per-block extraction split: ScalarE copies groups A (3 banks) + B
# (2 banks) = 64 regions as fp16; VectorE direct-reduces group D (1 bank,
# 11 regions) straight from PSUM. Group read times are sized so each
# group's PE refill hides under the other group's ScalarE read.

_STATE = None


def _build_nc(repeats=1, loop_n=None):
    import concourse.bacc as bacc
    import concourse.mybir as mybir
    import concourse.tile as tile

    f32 = mybir.dt.float32
    f32r = mybir.dt.float32r
    f16 = mybir.dt.float16
    AX = mybir.AxisListType
    OP = mybir.AluOpType
    ACT = mybir.ActivationFunctionType

    nc = bacc.Bacc("TRN2", target_bir_lowering=False, debug=False)

    L1 = nc.dram_tensor("l1", [5, RP], f32r, kind="ExternalInput")
    L2 = nc.dram_tensor("l2", [5, RP], f32r, kind="ExternalInput")
    R1 = nc.dram_tensor("r1", [5, CPAD], f32r, kind="ExternalInput")
    R2 = nc.dram_tensor("r2", [5, CPAD], f32r, kind="ExternalInput")
    EM = nc.dram_tensor("emat", [128, NB * R], f32, kind="ExternalInput")
    M1 = nc.dram_tensor("m1", [R, R], f32, kind="ExternalInput")
    M2 = nc.dram_tensor("m2", [R, R], f32, kind="ExternalInput")
    OUT = nc.dram_tensor("out", [1, 1], f32, kind="ExternalOutput")

    with tile.TileContext(nc) as tc:
        with (
            tc.tile_pool(name="io", bufs=1) as io,
            tc.tile_pool(name="hq", bufs=3) as hq,
            tc.tile_pool(name="t1p", bufs=2) as t1p,
            tc.tile_pool(name="t2p", bufs=2) as t2p,
            tc.tile_pool(name="mp", bufs=6) as mp,
            tc.tile_pool(name="fin", bufs=1) as fin,
            tc.tile_pool(name="psh", bufs=1, space="PSUM") as psh,
            tc.tile_pool(name="pss", bufs=1, space="PSUM") as pss,
        ):
            # pass-1 operands first: every DMA costs ~0.6us of HWDGE
            # dispatch, so load what the first block needs up front
            l1sb = io.tile([5, RP], f32r)
            nc.sync.dma_start(l1sb[:], L1[:])
            r2sb = io.tile([5, CPAD], f32r)
            nc.sync.dma_start(r2sb[:], R2[:])
            # remaining inputs go through the SWDGE queue in parallel with
            # the HWDGE loads the first blocks need
            l2sb = io.tile([5, RP], f32r)
            nc.gpsimd.dma_start(l2sb[:], L2[:])
            r1sb = io.tile([5, CPAD], f32r)
            nc.gpsimd.dma_start(r1sb[:], R1[:])
            emsb = io.tile([128, NB * R], f32)
            nc.gpsimd.dma_start(emsb[:], EM[:])
            m1sb = io.tile([R, R], f32)
            nc.gpsimd.dma_start(m1sb[:], M1[:])
            m2sb = io.tile([R, R], f32)
            nc.gpsimd.dma_start(m2sb[:], M2[:])
            ones = io.tile([R, 1], f32)
            nc.vector.memset(ones[:], 1.0)
            # warm the ACT spline-table cache while DMAs run, so the
            # implicit table load doesn't gate the first real Activation
            warm = io.tile([1, 1], f16)
            nc.scalar.activation(warm[:], ones[0:1, 0:1], ACT.Relu, bias=0.0)
            # warm the PE clock (HAM releases the throttle after ~3.4us of
            # sustained activity) with dummy matmuls on scratch data while
            # the input DMAs are in flight
            dmy32 = io.tile([5, 512], f32)
            nc.vector.memset(dmy32[:], 0.0)
            dmy = io.tile([5, 512], f32r)
            nc.vector.tensor_copy(dmy[:], dmy32[:])

            # 7 PSUM banks for H chunks (split into two tiles so the reader
            # of one group doesn't create a WAR hazard against refill of the
            # other) + 1 bank for the two accumulators
            hpsA = psh.tile([128, 3, 512], f32, name="hpsA")
            hpsB = psh.tile([128, 2, 512], f32, name="hpsB")
            hpsD = psh.tile([128, 1, 512], f32, name="hpsD")

            for _ in range(8):
                nc.tensor.matmul(
                    hpsA[:, 0, :],
                    lhsT=dmy[:, 0:128],
                    rhs=dmy[:],
                    start=True,
                    stop=True,
                    skip_group_check=True,
                )
            s = pss.tile([R, 2, R], f32)

            # slot groups: [0:4) and [4:7) ping-pong so PE can refill one
            # group while ScalarE/VectorE drains the other
            G0, G1 = (0, 4), (4, CH)
            LOOKAHEAD = 2  # defer E-matmul so PE never waits on DVE inline

            passes = [
                (l1sb, r2sb, 0),  # rows = p1 pts, cols = p2 pts
                (l2sb, r1sb, 1),  # rows = p2 pts, cols = p1 pts
            ]
            import contextlib

            if loop_n is not None:
                loop_cm = tc.For_i(
                    0, loop_n, 1, hint_engines=(mybir.EngineType.PE,)
                )
            else:
                loop_cm = contextlib.nullcontext()
            with loop_cm:
                for lsb, rsb, pi in passes * repeats:
                    pending = []

                    def flush_emm(upto):
                        while pending and pending[0][0] <= upto:
                            kk, mm_ = pending.pop(0)
                            nc.tensor.matmul(
                                s[:, pi, :],
                                lhsT=emsb[:, R * kk : R * (kk + 1)],
                                rhs=mm_[:, 0:R],
                                start=(kk == 0),
                                stop=(kk == NB - 1),
                                skip_group_check=True,
                            )

                    for k in range(NB):
                        # float32r streams at 1 cycle/row for N>=256 (plain fp32
                        # is 4 cycles/row); full 4-byte data, handled by the
                        # fused matmul weight-load path
                        lhsT = lsb[:, 128 * k : 128 * (k + 1)]
                        # D-chunk first: its PSUM bank frees early (VectorE
                        # reduce) and emitting it before the A-group keeps it
                        # out of the A-refill window on the ACT critical path
                        nc.tensor.matmul(
                            hpsD[:, 0, 0:CW],
                            lhsT=lhsT,
                            rhs=rsb[:, 2560:NR],
                            start=True,
                            stop=True,
                            skip_group_check=True,
                        )
                        for c in range(3):
                            nc.tensor.matmul(
                                hpsA[:, c, :],
                                lhsT=lhsT,
                                rhs=rsb[:, 512 * c : 512 * (c + 1)],
                                start=True,
                                stop=True,
                                skip_group_check=True,
                            )
                        for c in range(2):
                            nc.tensor.matmul(
                                hpsB[:, c, :],
                                lhsT=lhsT,
                                rhs=rsb[
                                    :, 1536 + 512 * c : 1536 + 512 * (c + 1)
                                ],
                                start=True,
                                stop=True,
                                skip_group_check=True,
                            )
                        h16 = hq.tile([128, 2560], f16, name="h16")
                        nc.scalar.activation(
                            h16[:, 0:1536],
                            hpsA[:].rearrange("p c w -> p (c w)"),
                            ACT.Relu,
                            bias=0.0,
                            scale=1.0,
                        )
                        nc.scalar.activation(
                            h16[:, 1536:2560],
                            hpsB[:].rearrange("p c w -> p (c w)"),
                            ACT.Relu,
                            bias=0.0,
                            scale=1.0,
                        )
                        rv = h16[:].rearrange("p (r m) -> p r m", m=M)
                        t1 = t1p.tile([128, 64, M // 2], f16, name="t1")
                        nc.vector.tensor_tensor(
                            out=t1[:],
                            in0=rv[:, :, 0 : M // 2],
                            in1=rv[:, :, M // 2 : M],
                            op=OP.min,
                        )
                        t2 = t2p.tile([128, 64, M // 4], f16, name="t2")
                        nc.vector.tensor_tensor(
                            out=t2[:],
                            in0=t1[:, :, 0 : M // 4],
                            in1=t1[:, :, M // 4 : M // 2],
                            op=OP.min,
                        )
                        mm = mp.tile([128, R], f32, name="mm")
                        nc.vector.tensor_reduce(
                            out=mm[:, 0:64], in_=t2[:], axis=AX.X, op=OP.min
                        )
                        nc.vector.tensor_reduce(
                            out=mm[:, 64:R],
                            in_=hpsD[:, 0, 0:CW].rearrange(
                                "p (u m) -> p u m", m=M
                            ),
                            axis=AX.X,
                            op=OP.min,
                        )
                        pending.append((k, mm))
                        flush_emm(k - LOOKAHEAD)
                    flush_emm(NB)


            u1 = fin.tile([R, R], f32)
            nc.vector.tensor_tensor(
                out=u1[:], in0=s[:, 0, :], in1=m1sb[:], op=OP.mult
            )
            u2 = fin.tile([R, R], f32)
            nc.vector.tensor_tensor(
                out=u2[:], in0=s[:, 1, :], in1=m2sb[:], op=OP.mult
            )
            us = fin.tile([R, R], f32)
            nc.vector.tensor_tensor(out=us[:], in0=u1[:], in1=u2[:], op=OP.add)
            rs = fin.tile([R, 1], f32)
            nc.vector.tensor_reduce(out=rs[:], in_=us[:], axis=AX.X, op=OP.add)
            # partition-direction sum via PE: [1,1] = ones^T @ rs
            nc.tensor.matmul(
                s[0:1, 0, 0:1],
                lhsT=ones[:],
                rhs=rs[:],
                start=True,
                stop=True,
                skip_group_check=True,
            )
            res = fin.tile([1, 1], f32)
            nc.scalar.mul(res[:], s[0:1, 0, 0:1], 1.0 / M)
            nc.sync.dma_start(OUT[:], res[:])

    nc.compile()
    return nc


def _build_runner(nc):
    import jax
    import numpy as _np
    from jax.experimental.shard_map import shard_map
    from jax.sharding import Mesh, PartitionSpec

    import concourse.mybir as mybir
    from concourse import bass2jax

    bass2jax.install_neuronx_cc_hook()

    pname = nc.partition_id_tensor.name if nc.partition_id_tensor else None
    in_names, out_names, out_avals, out_shapes = [], [], [], []
    for alloc in nc.m.functions[0].allocations:
        if not isinstance(alloc, mybir.MemoryLocationSet):
            continue
        name = alloc.memorylocations[0].name
        if alloc.kind == "ExternalInput":
            if name != pname:
                in_names.append(name)
        elif alloc.kind == "ExternalOutput":
            out_names.append(name)
            shape = tuple(alloc.tensor_shape)
            dtype = mybir.dt.np(alloc.dtype)
            out_avals.append(jax.core.ShapedArray(shape, dtype))
            out_shapes.append((shape, dtype))
    n_params = len(in_names)
    n_outs = len(out_names)
    all_names = in_names + out_names

    def _body(*args):
        operands = list(args)
        names = list(all_names)
        if pname is not None:
            operands.append(bass2jax.partition_id_tensor())
            names.append(pname)
        outs = bass2jax._bass_exec_p.bind(
            *operands,
            out_avals=tuple(out_avals),
            in_names=tuple(names),
            out_names=tuple(out_names),
            lowering_input_output_aliases=(),
            sim_require_finite=True,
            sim_require_nnan=True,
            nc=nc,
        )
        return tuple(outs)

    n_cores = B
    devices = jax.devices()[:n_cores]
    mesh = Mesh(_np.asarray(devices), ("core",))
    in_specs = (PartitionSpec("core"),) * (n_params + n_outs)
    out_specs = (PartitionSpec("core"),) * n_outs
    donate = tuple(range(n_params, n_params + n_outs))
    sharded = jax.jit(
        shard_map(
            _body, mesh=mesh, in_specs=in_specs, out_specs=out_specs,
            check_rep=False,
        ),
        donate_argnums=donate,
        keep_unused=True,
    )

    def run(in_maps):
        concat_in = [
            _np.ascontiguousarray(
                _np.concatenate([in_maps[c][name] for c in range(n_cores)], 0)
            )
            for name in in_names
        ]
        concat_zeros = [
            _np.zeros((n_cores * sh[0], *sh[1:]), dt) for sh, dt in out_shapes
        ]
        out_arrs = jax.block_until_ready(sharded(*concat_in, *concat_zeros))
        return [
            {
                name: _np.asarray(out_arrs[i]).reshape(
                    n_cores, *out_shapes[i][0]
                )[c]
                for i, name in enumerate(out_names)
            }
            for c in range(n_cores)
        ]

    return run


def _get_state():
    global _STATE
    if _STATE is None:
        nc = _build_nc()
        run = _build_runner(nc)
        _STATE = (nc, run)
    return _STATE


def make_in_maps(v1, v2, cmap, rid_to_vid):
    v1 = np.ascontiguousarray(np.asarray(v1), dtype=np.float32)
    v2 = np.ascontiguousarray(np.asarray(v2), dtype=np.float32)
    cmap = np.asarray(cmap)
    flat = np.asarray(rid_to_vid).astype(np.int64).ravel()  # [3000]

    rows = np.arange(RP)
    valid = rows < NR
    p_idx = rows % 128
    k_idx = rows // 128
    reg = rows // M
    emat = np.zeros((128, NB * R), np.float32)
    emat[p_idx[valid], k_idx[valid] * R + reg[valid]] = 1.0

    in_maps = []
    for b in range(B):
        p1 = v1[b][flat]  # [3000, 3]
        p2 = v2[b][flat]
        n1 = (p1 * p1).sum(-1)
        n2 = (p2 * p2).sum(-1)

        l1 = np.zeros((5, RP), np.float32)
        l1[0:3, :NR] = -2.0 * p1.T
        l1[3, :NR] = 1.0
        l1[4, :NR] = n1
        l2 = np.zeros((5, RP), np.float32)
        l2[0:3, :NR] = -2.0 * p2.T
        l2[3, :NR] = 1.0
        l2[4, :NR] = n2
        r1 = np.zeros((5, CPAD), np.float32)
        r1[0:3, :NR] = p1.T
        r1[3, :NR] = n1
        r1[4, :NR] = 1.0
        r2 = np.zeros((5, CPAD), np.float32)
        r2[0:3, :NR] = p2.T
        r2[3, :NR] = n2
        r2[4, :NR] = 1.0

        m1 = (cmap[b] != 0).astype(np.float32)
        m2 = np.ascontiguousarray(m1.T)

        in_maps.append(
            {
                "l1": l1, "l2": l2, "r1": r1, "r2": r2,
                "emat": emat,
                "m1": m1, "m2": m2,
            }
        )
    return in_maps


def kernel(v1, v2, cmap, rid_to_vid):
    _, run = _get_state()
    in_maps = make_in_maps(v1, v2, cmap, rid_to_vid)
    results = run(in_maps)
    return np.array(
        [results[b]["out"][0, 0] for b in range(B)], dtype=np.float32
    )

